# revision 27
# baseline (speedup 1.0000x reference)
"""Trainium2 Bass kernel: Bahdanau local-p attention (B=32, S=2048, H=1024).

Sharding: data-parallel over batch. Each of the 8 cores processes B/8 = 4
batches end-to-end (weights replicated); no collectives.

Host-side prep (inside kernel(), numpy): inputs are cast to fp8-e4m3 and
packed into the exact SBUF layouts (partition-major, DoubleRow k-pair
interleave) in BOTH orientations — transposed [H, S] for the score matmul
and natural [S, H] for the context matmul — so the device does only
contiguous DMAs: no cast DMAs, no DMA transposes. The tiny h_t-only
projections (p_t, U_a h_t bias, the h_t half of the concat; 0.2% of
FLOPs) are precomputed on host in f32; all S-dimension work runs on
device.

Per-core dataflow (per batch):
  1. WH^T tiles [128, 512] = W_a-tile^T @ inT on PE, fp8 DoubleRow.
  2. tanh(WH^T + U_a h_t) on ACT (per-partition bias), fp8 out,
     hp-pairs interleaved for the DoubleRow v_a dot.
  3. score = v_a^T tanh(...) via M=1 fp8 DoubleRow matmuls (PE).
  4. softmax (minus-max) + gaussian window on DVE/ACT rows; the
     gaussian factor is computed at batch start (independent of score)
     so the post-score critical chain is short. Weights scaled by 2^16
     into fp8 range.
  5. context = w^T @ x on PE: weight row transposed on-PE into fp8
     columns (16-padded for the DoubleRow LDWEIGHTS step constraint),
     then fp8 DoubleRow matmuls against the natural-layout x copy,
     descaled on ACT, transposed back into combT. This whole block for
     batch b is emitted after batch b+1's first hp-groups so the
     in-order PE queue never waits on the softmax chain.
  6. final tanh([ctx, h_t] @ W_att) fp16 matmuls; the h_t half runs
     during batch 3's softmax, the ctx half interleaves with batch 3's
     context block.
"""

import math
from contextlib import ExitStack

import numpy as np

B, S, H, SIZE = 32, 2048, 1024, 1024
N_CORES = 8
BPC = B // N_CORES
P = 128
NB = 512
KT = H // P        # 8  k-tiles over H
K8 = H // (2 * P)  # 4  double-k-tiles (DoubleRow)
K8V = KT // 2      # 4  hp-pairs for the v_a dot
SQ = S // NB       # 4  s blocks
NT = S // P        # 16 s-chunks of 128
NT2 = S // (2 * P)  # 8 double-s-chunks (ctx DoubleRow)
KT2 = 2 * H // P   # 16 k-tiles over 2H (final projection)
NO = SIZE // NB    # 2  output blocks
WSCALE = 65536.0   # fp8 range scale for the softmax weights

_compiled = None


def _build(bpc=BPC, s=S, h=H, size=SIZE, debug=False):
    import concourse.bacc as bacc
    import concourse.mybir as mybir
    import concourse.tile as tile

    F32 = mybir.dt.float32
    F16 = mybir.dt.float16
    F8 = mybir.dt.float8e4
    AF = mybir.ActivationFunctionType
    ALU = mybir.AluOpType
    AX = mybir.AxisListType
    DR = mybir.MatmulPerfMode.DoubleRow

    denom = 2.0 * ((s // 2) / 2.0) ** 2
    inv_sq_denom = 1.0 / math.sqrt(denom)

    nc = bacc.Bacc("TRN2", target_bir_lowering=False, debug=debug)

    x8 = nc.dram_tensor("x8", [bpc, P, K8, 2, s], F8, kind="ExternalInput").ap()
    xn8 = nc.dram_tensor("xn8", [bpc, P, NT2, 2, h], F8,
                         kind="ExternalInput").ap()
    wa8 = nc.dram_tensor("wa8", [P, K8, 2, h], F8, kind="ExternalInput").ap()
    va8 = nc.dram_tensor("va8", [P, K8V, 2, 16], F8, kind="ExternalInput").ap()
    wtT_d = nc.dram_tensor("wtT", [P, KT, bpc], F32, kind="ExternalInput").ap()
    comb0 = nc.dram_tensor("comb0", [P, KT, bpc], F16,
                           kind="ExternalInput").ap()
    prow_d = nc.dram_tensor("prow", [1, bpc], F32, kind="ExternalInput").ap()
    watt16 = nc.dram_tensor("watt16", [P, KT2, size], F16,
                            kind="ExternalInput").ap()
    out = nc.dram_tensor("out", [bpc, size], F32, kind="ExternalOutput").ap()

    with tile.TileContext(nc) as tc, ExitStack() as ctx:
        sb = ctx.enter_context(tc.tile_pool(name="sb", bufs=1))
        ps = ctx.enter_context(tc.tile_pool(name="ps", bufs=1, space="PSUM"))
        dp = ctx.enter_context(tc.tile_pool(name="dram", bufs=2, space="DRAM"))

        inT_tiles = [None] * bpc
        xnt_tiles = [None] * bpc

        def emit_input_dma(b, chunked=False):
            inT = sb.tile([P, K8, 2, s], F8, name=f"inT_{b}", tag="big",
                          bufs=4)
            if chunked:
                nc.sync.dma_start(inT[:, 0:K8 // 2], x8[b, :, 0:K8 // 2])
                nc.sync.dma_start(inT[:, K8 // 2:], x8[b, :, K8 // 2:])
            else:
                nc.sync.dma_start(inT[:], x8[b])
            inT_tiles[b] = inT

        # ---- startup DMAs. Small tensors + weights on the ACT HWDGE
        # ring (wa chunked per k2 so hp0 can start on the first 256 KB);
        # the big inputs on the SP HWDGE ring.
        wa_sb = sb.tile([P, K8, 2, h], F8, name="wa_sb", tag="wa")
        nc.sync.dma_start(wa_sb[:], wa8[:])
        wtT = sb.tile([P, KT, bpc], F32, name="wtT", tag="wtT")
        nc.scalar.dma_start(wtT[:], wtT_d[:])
        va_sb = sb.tile([P, K8V, 2, 16], F8, name="va_sb", tag="va")
        nc.scalar.dma_start(va_sb[:], va8[:])
        p_row = sb.tile([1, bpc], F32, name="p_row", tag="p_row")
        nc.scalar.dma_start(p_row[:], prow_d[:])
        emit_input_dma(0)
        emit_input_dma(1)
        emit_input_dma(2)
        emit_input_dma(3)
        for b in range(bpc):
            xnt = sb.tile([P, NT2, 2, h], F8, name=f"xnt_{b}", tag="bigN",
                          bufs=2)
            nc.sync.dma_start(xnt[:], xn8[b])
            xnt_tiles[b] = xnt

        # ---- constants / persistent tiles ----
        ident1 = sb.tile([1, 1], F32, name="ident1", tag="ident1")
        nc.vector.memset(ident1[:], 1.0)
        pos_row = sb.tile([1, s], F16, name="pos_row", tag="pos")
        nc.gpsimd.iota(pos_row[:], pattern=[[1, s]], base=0,
                       channel_multiplier=0,
                       allow_small_or_imprecise_dtypes=True)
        # fp8 weight columns, 16-padded; cols 1-15 stay zero forever
        wTp = sb.tile([P, NT2, 2, 16], F8, name="wTp", tag="wTp")
        nc.vector.memset(wTp[:], 0.0)
        combT = sb.tile([P, KT2, bpc], F16, name="combT", tag="combT")

        watt_holder = [None]

        def emit_watt_dma():
            w16 = sb.tile([P, KT2, size], F16, name="watt_sb", tag="wend")
            nc.scalar.dma_start(w16[:], watt16[:])
            watt_holder[0] = w16
            # h_t half of combT (ctx half written later by ctx blocks)
            nc.scalar.dma_start(combT[:, KT:, :], comb0[:])

        # three-stage deferred ctx pipeline, keyed by due batch:
        #   due_wu[b+1]  - weight-row bounce DMAs (data ready before PE)
        #   due_pe[b+1]  - ctx matmuls + descale + ctx-row scratch write
        #   due_read[b+2]- ctx column read-back into combT (needed only
        #                  by the finale)
        due_wu, due_pe, due_read = {}, {}, {}
        ctxd_tiles = {}

        # ---- main batch loop ----
        for b in range(bpc):
            if b == 2 or bpc <= 2:
                emit_watt_dma()
            inT = inT_tiles[b]

            # gaussian row: independent of the score — compute during
            # the batch's matmuls, off the post-score critical chain.
            dr = sb.tile([1, s], F16, name=f"dr_{b}", tag="gA")
            nc.vector.tensor_scalar(dr[:], pos_row[:], p_row[0:1, b:b + 1],
                                    inv_sq_denom, op0=ALU.subtract,
                                    op1=ALU.mult)
            d2 = sb.tile([1, s], F16, name=f"d2_{b}", tag="gB")
            nc.vector.tensor_mul(d2[:], dr[:], dr[:])
            gr = sb.tile([1, s], F16, name=f"gr_{b}", tag="gA")
            nc.scalar.activation(gr[:], d2[:], AF.Exp, scale=-1.0)

            # sc_ps allocated lazily at the first v_a matmul so the pool
            # rotation sequences them AFTER the deferred tiles of the
            # previous batch's ctx block.
            sc_ps = []

            def emit_va_mms(j, tanh_tiles):
                if j == 0:
                    sc_ps[:] = [ps.tile([16, NB], F32, name=f"sc_{b}_{q}",
                                        tag="sc", bufs=4) for q in range(SQ)]
                for q in range(SQ):
                    nc.tensor.matmul(sc_ps[q][:], va_sb[:, j],
                                     tanh_tiles[q][:],
                                     start=(j == 0), stop=(j == K8V - 1),
                                     perf_mode=DR, skip_group_check=True)

            # v_a matmuls run one hp-pair behind the main matmuls so the
            # PE never waits on ACT's tanh.
            pend2 = None
            for hp in range(KT):
                wh_ps = [ps.tile([P, NB], F32, name=f"wh_{b}_{hp}_{q}",
                                 tag="wh", bufs=4) for q in range(SQ)]
                for k2 in range(K8):
                    lhsT = wa_sb[:, k2, :, hp * P:(hp + 1) * P]
                    for q in range(SQ):
                        nc.tensor.matmul(
                            wh_ps[q][:], lhsT,
                            inT[:, k2, :, q * NB:(q + 1) * NB],
                            start=(k2 == 0), stop=(k2 == K8 - 1),
                            perf_mode=DR, skip_group_check=True)
                if hp == 2:
                    if b in due_read:
                        due_read.pop(b)()
                    if b in due_wu:
                        due_wu.pop(b)()
                if hp == 4 and b in due_pe:
                    due_pe.pop(b)()
                if hp % 2 == 0 and hp >= 2:
                    emit_va_mms(hp // 2 - 1, pend2)
                if hp % 2 == 0:
                    pend2 = [sb.tile([P, 2, NB], F8, name=f"th_{b}_{hp}_{q}",
                                     tag="tanh", bufs=8) for q in range(SQ)]
                for q in range(SQ):
                    nc.scalar.activation(pend2[q][:, hp % 2, :], wh_ps[q][:],
                                         AF.Tanh, bias=wtT[:, hp, b:b + 1])
            emit_va_mms(K8V - 1, pend2)

            # ---- softmax: short post-score chain. No minus-max (scores
            # are bounded by |v_a^T tanh| <= ~1.6, exp cannot overflow);
            # exp reads straight from PSUM with fused partial sums.
            e1 = sb.tile([1, s], F16, name=f"e1_{b}", tag="e1")
            se4 = sb.tile([1, SQ], F32, name=f"se4_{b}", tag="se4", bufs=2)
            for q in range(SQ):
                nc.scalar.activation(e1[0:1, q * NB:(q + 1) * NB],
                                     sc_ps[q][0:1, :], AF.Exp,
                                     accum_out=se4[0:1, q:q + 1])
            se = sb.tile([1, 1], F32, name=f"se_{b}", tag="se", bufs=2)
            nc.vector.tensor_reduce(se[:], se4[:], axis=AX.X, op=ALU.add)
            rr = sb.tile([1, 1], F32, name=f"rr_{b}", tag="rr", bufs=2)
            nc.vector.reciprocal(rr[:], se[:])
            rrS = sb.tile([1, 1], F32, name=f"rrS_{b}", tag="rrS", bufs=2)
            nc.vector.tensor_scalar_mul(rrS[:], rr[:], WSCALE)
            wu = sb.tile([1, s], F32, name=f"wu_{b}", tag="wu")
            nc.vector.scalar_tensor_tensor(wu[:], e1[:], rrS[0:1, 0:1], gr[:],
                                           op0=ALU.mult, op1=ALU.mult)

            def make_wu_read(b, wud):
                def emit():
                    wuTf = sb.tile([P, NT], F32, name=f"wuTf_{b}",
                                   tag="wuTf", bufs=2)
                    nc.scalar.dma_start(
                        wuTf[:],
                        wud[0:1, :].rearrange("o (t p) -> p t o", p=P))
                    nc.vector.tensor_copy(
                        wTp[:, :, :, 0],
                        wuTf[:, :].rearrange("p (t2 i) -> p t2 i", i=2))
                return emit

            def emit_ctx_mms(b, xnt):
                pc = [ps.tile([16, NB], F32, name=f"pc_{b}_{hh}",
                              tag="sc", bufs=4) for hh in range(2)]
                for t2 in range(NT2):
                    lhsT = wTp[:, t2]
                    for hh in range(2):
                        nc.tensor.matmul(
                            pc[hh][:], lhsT,
                            xnt[:, t2, :, hh * NB:(hh + 1) * NB],
                            start=(t2 == 0), stop=(t2 == NT2 - 1),
                            perf_mode=DR, skip_group_check=True)
                ctxrow = sb.tile([1, h], F32, name=f"ctxr_{b}",
                                 tag="ctxrow", bufs=2)
                for hh in range(2):
                    nc.scalar.activation(
                        ctxrow[0:1, hh * NB:(hh + 1) * NB],
                        pc[hh][0:1, :], AF.Copy, scale=1.0 / WSCALE)
                return ctxrow

            def make_ctx_pe(b, xnt):
                def emit():
                    ctxrow = emit_ctx_mms(b, xnt)
                    ctxd = dp.tile([1, h], F32, name=f"ctxd_{b}",
                                   tag="ctxd")
                    nc.gpsimd.dma_start(ctxd[:], ctxrow[:])
                    ctxd_tiles[b] = ctxd
                return emit

            def make_ctx_read(b):
                def emit():
                    ctxT = sb.tile([P, KT], F32, name=f"ctxT_{b}",
                                   tag="ctxT", bufs=2)
                    nc.scalar.dma_start(
                        ctxT[:],
                        ctxd_tiles[b][0:1, :].rearrange(
                            "o (k p) -> p k o", p=P))
                    nc.vector.tensor_copy(combT[:, 0:KT, b:b + 1],
                                          ctxT[:, :].rearrange(
                                              "p k -> p k ()"))
                return emit

            def make_ctx_tail(b, xnt, wu, final_cb):
                # batch 3: all-PE path (transposes), lowest tail latency
                def emit():
                    for t in range(NT):
                        pT = ps.tile([P, 1], F32, name=f"pT_{b}_{t}",
                                     tag="sc", bufs=4)
                        nc.tensor.transpose(pT[:],
                                            wu[0:1, t * P:(t + 1) * P],
                                            ident1[:])
                        nc.vector.tensor_copy(wTp[:, t // 2, t % 2, 0:1],
                                              pT[:])
                    ctxrow = emit_ctx_mms(b, xnt)
                    for kk in range(KT):
                        pC = ps.tile([P, 1], F32, name=f"pC_{b}_{kk}",
                                     tag="sc", bufs=4)
                        nc.tensor.transpose(
                            pC[:], ctxrow[0:1, kk * P:(kk + 1) * P],
                            ident1[:])
                        nc.vector.tensor_copy(combT[:, kk, b:b + 1],
                                              pC[:])
                        final_cb(kk)
                return emit

            if b < bpc - 1:
                # scratch write issued now (gpsimd queue is idle; runs as
                # soon as wu's semaphore fires) so the deferred read
                # never blocks the ACT queue
                wud = dp.tile([1, s], F32, name=f"wud_{b}", tag="wud")
                nc.gpsimd.dma_start(wud[:], wu[:])
                due_wu[b + 1] = make_wu_read(b, wud)
                due_pe[b + 1] = make_ctx_pe(b, xnt_tiles[b])
                due_read[b + 2] = make_ctx_read(b)

        # ---- final projection: tanh([ctx, h_t] @ W_att) ----
        watt_sb = watt_holder[0]
        pfs = [ps.tile([bpc, NB], F32, name=f"pf_{n2}", tag="wh", bufs=4)
               for n2 in range(NO)]

        def emit_final(kk):
            for n2 in range(NO):
                nc.tensor.matmul(pfs[n2][:], combT[:, kk, :],
                                 watt_sb[:, kk, n2 * NB:(n2 + 1) * NB],
                                 start=(kk == KT), stop=(kk == KT - 1),
                                 skip_group_check=True)

        # h_t half runs during batch 3's softmax; ctx half interleaves
        # with batch 3's context block.
        if bpc in due_read:
            due_read.pop(bpc)()
        for kk in range(KT, KT2):
            emit_final(kk)
        make_ctx_tail(bpc - 1, xnt_tiles[bpc - 1], wu, emit_final)()

        outsb = sb.tile([bpc, size], F32, name="outsb", tag="outsb")
        for n2 in range(NO):
            nc.scalar.activation(outsb[:, n2 * NB:(n2 + 1) * NB], pfs[n2][:],
                                 AF.Tanh)
        nc.scalar.dma_start(out[:], outsb[:])

    nc.compile()
    return nc


def build_in_maps(inputs):
    """Host-side packing: shard batch over cores, transpose/cast/pack
    inputs and weights into the exact device layouts, and precompute the
    tiny h_t-only projections in f32."""
    import ml_dtypes

    F8 = ml_dtypes.float8_e4m3
    F16 = np.float16

    x = np.asarray(inputs["inputs"], dtype=np.float32)
    W_p = np.asarray(inputs["W_p"], np.float32)
    v_p = np.asarray(inputs["v_p"], np.float32)
    W_a = np.asarray(inputs["W_a"], np.float32)
    U_a = np.asarray(inputs["U_a"], np.float32)
    v_a = np.asarray(inputs["v_a"], np.float32)
    W_att = np.asarray(inputs["W_att"], np.float32)

    xf8 = x.astype(F8)
    # transposed copy [B, P, K8, 2, S] (h partition-major, k-pair pairs)
    x8 = np.ascontiguousarray(
        xf8.transpose(0, 2, 1).reshape(B, K8, 2, P, S).transpose(0, 3, 1, 2, 4))
    # natural copy [B, P, NT2, 2, H] (s partition-major, s-pair pairs)
    xn8 = np.ascontiguousarray(
        xf8.reshape(B, NT2, 2, P, H).transpose(0, 3, 1, 2, 4))

    h_t = x[:, -1, :]                                   # [B, H] f32
    wt = h_t @ U_a                                      # [B, H]
    p_t = 1.0 / (1.0 + np.exp(-(np.tanh(h_t @ W_p) @ v_p))) * S  # [B, 1]

    wa8 = np.ascontiguousarray(
        W_a.astype(F8).reshape(K8, 2, P, H).transpose(2, 0, 1, 3))
    va8 = np.zeros((P, K8V, 2, 16), dtype=F8)
    va8[:, :, :, 0] = v_a[:, 0].reshape(K8V, 2, P).transpose(2, 0, 1).astype(F8)
    watt16 = np.ascontiguousarray(
        W_att.astype(F16).reshape(KT2, P, SIZE).transpose(1, 0, 2))

    in_maps = []
    for i in range(N_CORES):
        sl = slice(i * BPC, (i + 1) * BPC)
        wtT = np.ascontiguousarray(
            wt[sl].T.reshape(KT, P, BPC).transpose(1, 0, 2))
        comb0 = np.ascontiguousarray(
            h_t[sl].T.reshape(KT, P, BPC).transpose(1, 0, 2).astype(F16))
        prow = np.ascontiguousarray(p_t[sl].T.astype(np.float32))
        in_maps.append({
            "x8": x8[sl], "xn8": xn8[sl], "wa8": wa8, "va8": va8,
            "wtT": wtT, "comb0": comb0, "prow": prow, "watt16": watt16,
        })
    return in_maps


def kernel(**inputs):
    global _compiled
    from concourse import bass_utils

    if _compiled is None:
        _compiled = _build()

    in_maps = build_in_maps(inputs)
    res = bass_utils.run_bass_kernel_spmd(_compiled, in_maps,
                                          list(range(N_CORES)))
    return np.concatenate([res.results[i]["out"] for i in range(N_CORES)],
                          axis=0).astype(np.float32)
